# revision 5
# baseline (speedup 1.0000x reference)
"""Trainium2 Bass kernel for nn_MultiHeadAttention (B=4, L=S=2048, D=1024, H=16, causal).

Sharding: 8 cores = 4 batches x 2 head-groups (8 heads each).
Per core: project its batch's q/k/v against its group's weight slices,
causal attention for 8 heads, output-projection against Wo column slice.
Host sums the 2 partial outputs per batch (tensor-parallel reduce).

All matmuls in bf16 with fp32 PSUM accumulation.
Layout: activations transposed on-chip ([D, tokens]) via cast-DMA + DMA-transpose.
"""

import sys

if "/opt/trn_rl_repo" not in sys.path:
    sys.path.insert(0, "/opt/trn_rl_repo")

import numpy as np
import ml_dtypes

BF16 = ml_dtypes.bfloat16

# Problem constants (hardcoded per harness contract)
B, L, D, H = 4, 2048, 1024, 16
HD = D // H              # 64
NCORES = 8
GROUPS = 2               # head-groups (tensor parallel)
HG = H // GROUPS         # 8 heads per group
DG = HG * HD             # 512 out-dim per group

# Full-size device config
FULL_CFG = dict(T=L, DM=D, DG=DG)


def emit_mha(tc, aps, cfg):
    """Emit the per-core MHA program into TileContext tc.

    aps: dict of bass APs: xq, xk, xv, wq, wk, wv, wo, maskt (inputs), y (output)
    cfg: dict(T, DM, DG)
    """
    import concourse.bass as bass
    from concourse import mybir

    nc = tc.nc
    f32 = mybir.dt.float32
    bf16 = mybir.dt.bfloat16
    Exp = mybir.ActivationFunctionType.Exp

    T, DM, DG_ = cfg["T"], cfg["DM"], cfg["DG"]
    TB = 128                  # s/l block
    LCH = min(512, T)         # l-chunk (moving-dim)
    nDch = DM // 128          # D chunks (contraction)
    nTt = T // TB             # token tiles
    nLch = T // LCH           # l-chunks
    nMask = LCH // TB         # diagonal mask tiles
    nPair = DG_ // 128        # head pairs (2 heads of 64 per pair)
    OCH = min(512, DM)        # Wo output chunk
    nOch = DM // OCH          # output chunks for Wo
    SCALE = 1.0 / np.sqrt(HD)

    import contextlib

    ctx = contextlib.ExitStack()
    with ctx:
        dram = ctx.enter_context(tc.tile_pool(name="dram", bufs=1, space="DRAM"))
        wpool = ctx.enter_context(tc.tile_pool(name="wts", bufs=1))
        xt_pool = ctx.enter_context(tc.tile_pool(name="xt", bufs=nDch))
        qkv_pool = ctx.enter_context(tc.tile_pool(name="qkv", bufs=1))
        pt_pool = ctx.enter_context(tc.tile_pool(name="pt", bufs=4))
        ctxt_pool = ctx.enter_context(tc.tile_pool(name="ctxt", bufs=1))
        small = ctx.enter_context(tc.tile_pool(name="small", bufs=4))
        outsb_pool = ctx.enter_context(tc.tile_pool(name="outsb", bufs=3))
        # PSUM: st 2x2 banks + ctx 2 + sums 1 + proj 1 = 8 banks
        st_ps = ctx.enter_context(tc.tile_pool(name="st_ps", bufs=2, space="PSUM"))
        ctx_ps_pool = ctx.enter_context(tc.tile_pool(name="ctx_ps", bufs=2, space="PSUM"))
        sums_ps_pool = ctx.enter_context(tc.tile_pool(name="sums_ps", bufs=1, space="PSUM"))
        proj_ps = ctx.enter_context(tc.tile_pool(name="proj_ps", bufs=1, space="PSUM"))

        # ---- constants ----
        ones = wpool.tile([128, 1], bf16, tag="ones")
        nc.vector.memset(ones[:], 1.0)
        masks = []
        for r in range(nMask):
            mt = wpool.tile([TB, 2 * LCH], bf16, tag=f"mask{r}")
            nc.sync.dma_start(out=mt[:], in_=aps["maskt"][r])
            masks.append(mt)

        # ---- stage inputs to bf16 DRAM (SWDGE cast) ----
        wqb = dram.tile([DG_, DM], bf16, tag="wqb")
        wkb = dram.tile([DG_, DM], bf16, tag="wkb")
        wvb = dram.tile([DG_, DM], bf16, tag="wvb")
        wob = dram.tile([DM, DG_], bf16, tag="wob")
        nc.gpsimd.dma_start(out=wqb[:], in_=aps["wq"])
        nc.gpsimd.dma_start(out=wkb[:], in_=aps["wk"])
        nc.gpsimd.dma_start(out=wvb[:], in_=aps["wv"])
        nc.gpsimd.dma_start(out=wob[:], in_=aps["wo"])
        vb = dram.tile([T, DM], bf16, tag="vb")
        qb = dram.tile([T, DM], bf16, tag="qb")
        kb = dram.tile([T, DM], bf16, tag="kb")
        nc.gpsimd.dma_start(out=vb[:], in_=aps["xv"])
        nc.gpsimd.dma_start(out=qb[:], in_=aps["xq"])
        nc.gpsimd.dma_start(out=kb[:], in_=aps["xk"])

        # ---- weight transposes: w*T[c] = W[:, 128c:128c+128].T -> [128, DG] ----
        def wtrans(dst_tag, src):
            tiles = []
            for c in range(nDch):
                t = wpool.tile([128, DG_], bf16, tag=f"{dst_tag}{c}")
                nc.sync.dma_start(out=t[:], in_=src[:, c * 128:(c + 1) * 128], transpose=True)
                tiles.append(t)
            return tiles

        wqT = wtrans("wqT", wqb)
        wkT = wtrans("wkT", wkb)
        wvT = wtrans("wvT", wvb)
        woT = []
        for c in range(DG_ // 128):
            t = wpool.tile([128, DM], bf16, tag=f"woT{c}")
            nc.sync.dma_start(out=t[:], in_=wob[:, c * 128:(c + 1) * 128], transpose=True)
            woT.append(t)

        def xtrans(src):
            tiles = []
            for c in range(nDch):
                t = xt_pool.tile([128, T], bf16, tag="xt")
                nc.sync.dma_start(out=t[:], in_=src[:, c * 128:(c + 1) * 128], transpose=True)
                tiles.append(t)
            return tiles

        # ---- V projection: V[st] [128, DG] natural (s on partitions) ----
        vT = xtrans(vb)
        V = []
        for st in range(nTt):
            ps = proj_ps.tile([128, min(512, DG_)], f32, tag="proj")
            # DG_ <= 512 assumed (one N chunk)
            for c in range(nDch):
                nc.tensor.matmul(ps[:], lhsT=vT[c][:, st * TB:(st + 1) * TB],
                                 rhs=wvT[c][:], start=(c == 0), stop=(c == nDch - 1))
            vt = qkv_pool.tile([128, DG_], bf16, tag=f"V{st}")
            nc.vector.tensor_copy(vt[:], ps[:])
            V.append(vt)

        # ---- QT projection: QT[m] [128, T] (dout on partitions) ----
        qT = xtrans(qb)
        QT = []
        for m in range(nPair):
            qt = qkv_pool.tile([128, T], bf16, tag=f"QT{m}")
            for n in range(nLch):
                ps = proj_ps.tile([128, LCH], f32, tag="proj")
                for c in range(nDch):
                    nc.tensor.matmul(ps[:], lhsT=wqT[c][:, m * 128:(m + 1) * 128],
                                     rhs=qT[c][:, n * LCH:(n + 1) * LCH],
                                     start=(c == 0), stop=(c == nDch - 1))
                nc.vector.tensor_copy(qt[:, n * LCH:(n + 1) * LCH], ps[:])
            QT.append(qt)

        # ---- K transposes (KT[p] projected lazily per pair) ----
        kT = xtrans(kb)

        ctxT = [[None] * nLch for _ in range(nPair)]
        for p in range(nPair):
            # KT[p] projection
            kt = qkv_pool.tile([128, T], bf16, tag=f"KT{p}")
            for n in range(nLch):
                ps = proj_ps.tile([128, LCH], f32, tag="proj")
                for c in range(nDch):
                    nc.tensor.matmul(ps[:], lhsT=wkT[c][:, p * 128:(p + 1) * 128],
                                     rhs=kT[c][:, n * LCH:(n + 1) * LCH],
                                     start=(c == 0), stop=(c == nDch - 1))
                nc.vector.tensor_copy(kt[:, n * LCH:(n + 1) * LCH], ps[:])
            qt = QT[p]

            # attention for this pair of heads
            for i in range(nLch):
                nsb = (i + 1) * (LCH // TB)
                cps = ctx_ps_pool.tile([128, LCH], f32, tag="ctx")
                sps = sums_ps_pool.tile([33, LCH], f32, tag="sums")
                for j in range(nsb):
                    sp = st_ps.tile([128, 2 * LCH], f32, tag="st")
                    nc.tensor.matmul(sp[:, 0:LCH],
                                     lhsT=kt[0:64, j * TB:(j + 1) * TB],
                                     rhs=qt[0:64, i * LCH:(i + 1) * LCH],
                                     start=True, stop=True)
                    nc.tensor.matmul(sp[:, LCH:2 * LCH],
                                     lhsT=kt[64:128, j * TB:(j + 1) * TB],
                                     rhs=qt[64:128, i * LCH:(i + 1) * LCH],
                                     start=True, stop=True)
                    pt = pt_pool.tile([128, 2 * LCH], bf16, tag="pt")
                    nc.scalar.activation(pt[:], sp[:], Exp, scale=float(SCALE))
                    r = j - (LCH // TB) * i
                    if r >= 0:
                        nc.vector.tensor_mul(pt[:], pt[:], masks[r][:])
                    st = (j == 0)
                    en = (j == nsb - 1)
                    nc.tensor.matmul(cps[0:64, :], lhsT=V[j][:, p * 128:p * 128 + 64],
                                     rhs=pt[:, 0:LCH], start=st, stop=en,
                                     skip_group_check=True)
                    nc.tensor.matmul(cps[64:128, :], lhsT=V[j][:, p * 128 + 64:p * 128 + 128],
                                     rhs=pt[:, LCH:2 * LCH], start=st, stop=en,
                                     skip_group_check=True)
                    nc.tensor.matmul(sps[0:1, :], lhsT=ones[:], rhs=pt[:, 0:LCH],
                                     start=st, stop=en, skip_group_check=True)
                    nc.tensor.matmul(sps[32:33, :], lhsT=ones[:], rhs=pt[:, LCH:2 * LCH],
                                     start=st, stop=en, skip_group_check=True)
                # normalize: ctxT_sb = cps * (1/sums) broadcast over partitions.
                # partition_broadcast only works from src base-partition 0 on HW,
                # so gather both recips to partition 0 first (lane-aligned DVE
                # recips, then a small partition-moving DMA).
                rec = small.tile([33, LCH], f32, tag="rec")
                nc.vector.reciprocal(rec[0:1, :], sps[0:1, :])
                nc.vector.reciprocal(rec[32:33, :], sps[32:33, :])
                rec01 = small.tile([1, 2 * LCH], f32, tag="rec01")
                nc.sync.dma_start(out=rec01[0:1, 0:LCH], in_=rec[0:1, :])
                nc.sync.dma_start(out=rec01[0:1, LCH:2 * LCH], in_=rec[32:33, :])
                rb = small.tile([128, 2 * LCH], f32, tag="rb")
                nc.gpsimd.partition_broadcast(rb[:], rec01[0:1, :])
                ct = ctxt_pool.tile([128, LCH], bf16, tag=f"ctxT{p}_{i}")
                nc.vector.tensor_mul(ct[0:64, :], cps[0:64, :], rb[0:64, 0:LCH])
                nc.vector.tensor_mul(ct[64:128, :], cps[64:128, :], rb[64:128, LCH:2 * LCH])
                ctxT[p][i] = ct

        # ---- Wo: y[lt*128:, :] = ctx @ WoT ----
        for lt in range(nTt):
            osb = outsb_pool.tile([128, DM], f32, tag="osb")
            for oc in range(nOch):
                ps = proj_ps.tile([128, OCH], f32, tag="proj")
                for dc in range(nPair):
                    lhsT = ctxT[dc][lt // nMask][:, (lt % nMask) * TB:(lt % nMask) * TB + TB]
                    nc.tensor.matmul(ps[:], lhsT=lhsT,
                                     rhs=woT[dc][:, oc * OCH:(oc + 1) * OCH],
                                     start=(dc == 0), stop=(dc == nPair - 1))
                nc.vector.tensor_copy(osb[:, oc * OCH:(oc + 1) * OCH], ps[:])
            nc.sync.dma_start(out=aps["y"][lt * TB:(lt + 1) * TB, :], in_=osb[:])


def make_mask_tiles(cfg):
    T, LCH, TB = cfg["T"], min(512, cfg["T"]), 128
    nMask = LCH // TB
    f = np.arange(2 * LCH) % LCH
    p = np.arange(TB)
    tiles = []
    for r in range(nMask):
        m = (f[None, :] >= (TB * r + p)[:, None]).astype(np.float32)
        tiles.append(m)
    return np.stack(tiles).astype(BF16)


def build_nc(cfg):
    """Build and compile the per-core Bass program. Returns (nc, input_names)."""
    import concourse.bacc as bacc
    import concourse.tile as tile
    from concourse import mybir

    T, DM, DG_ = cfg["T"], cfg["DM"], cfg["DG"]
    LCH = min(512, T)
    nMask = LCH // 128

    nc = bacc.Bacc("TRN2", target_bir_lowering=False, debug=False)
    f32 = mybir.dt.float32
    bf16 = mybir.dt.bfloat16
    aps = {}
    for nm, shape, dt in [
        ("xq", [T, DM], f32), ("xk", [T, DM], f32), ("xv", [T, DM], f32),
        ("wq", [DG_, DM], f32), ("wk", [DG_, DM], f32), ("wv", [DG_, DM], f32),
        ("wo", [DM, DG_], f32),
        ("maskt", [nMask, 128, 2 * LCH], bf16),
    ]:
        aps[nm] = nc.dram_tensor(nm, shape, dt, kind="ExternalInput").ap()
    aps["y"] = nc.dram_tensor("y", [T, DM], f32, kind="ExternalOutput").ap()

    with tile.TileContext(nc) as tc:
        emit_mha(tc, aps, cfg)
    nc.compile()
    return nc


_CACHE = {}


def _get_nc():
    if "nc" not in _CACHE:
        _CACHE["nc"] = build_nc(FULL_CFG)
    return _CACHE["nc"]


def shard_inputs(q, k, v, Wq, Wk, Wv, Wo):
    """Build the per-core input maps (8 cores = 4 batches x 2 groups)."""
    maskt = make_mask_tiles(FULL_CFG)
    in_maps = []
    for core in range(NCORES):
        b, g = divmod(core, GROUPS)
        rows = slice(g * DG, (g + 1) * DG)
        in_maps.append({
            "xq": np.ascontiguousarray(q[b]),
            "xk": np.ascontiguousarray(k[b]),
            "xv": np.ascontiguousarray(v[b]),
            "wq": np.ascontiguousarray(Wq[rows]),
            "wk": np.ascontiguousarray(Wk[rows]),
            "wv": np.ascontiguousarray(Wv[rows]),
            "wo": np.ascontiguousarray(Wo[:, rows]),
            "maskt": maskt,
        })
    return in_maps


def kernel(q, k, v, mask, Wq, Wk, Wv, Wo):
    from concourse import bass_utils

    q = np.asarray(q, dtype=np.float32)
    k = np.asarray(k, dtype=np.float32)
    v = np.asarray(v, dtype=np.float32)
    Wq = np.asarray(Wq, dtype=np.float32)
    Wk = np.asarray(Wk, dtype=np.float32)
    Wv = np.asarray(Wv, dtype=np.float32)
    Wo = np.asarray(Wo, dtype=np.float32)

    nc = _get_nc()
    in_maps = shard_inputs(q, k, v, Wq, Wk, Wv, Wo)
    res = bass_utils.run_bass_kernel_spmd(nc, in_maps, core_ids=list(range(NCORES)))
    out = np.zeros((B, L, D), dtype=np.float32)
    for core in range(NCORES):
        b = core // GROUPS
        out[b] += res.results[core]["y"]
    return out
